# revision 33
# baseline (speedup 1.0000x reference)
"""Bass/Trainium2 kernel for nn_Attention_66297115181568 (sparse_attention).

Strategy: head-parallel across 8 NeuronCores. Core h computes head h
end-to-end; the host sums the 8 partial (512, 512) outputs (the
tensor-parallel all-reduce done at unshard time) and adds bo.

The dominant cost is streaming rel_pos (67MB/core in f32). Two changes
vs the v1 kernel:
  1. rel_pos is cast to fp16 and transposed to (i-tile, pair, k, j)
     layout ON THE HOST: DMA bytes halve to 33.5MB/core (~94us at the
     ~356GB/s per-core HBM ceiling).
  2. The rel contraction relterm[i,j] = sum_d q'[i,d]*rel[i,j,d] moves
     from DVE/ACT/GpSimd onto the idle tensor engine: for each pair of
     rows (i0=base+p, i1=base+64+p) a 128x128 stationary holds q'[i0]
     in column p (k rows 64:128) and q'[i1] in column p+64 (k rows
     0:64), zeros elsewhere; the moving operand is the (128, 512) rel
     tile with both rows' d-vectors stacked on partitions. 64 such
     matmuls accumulate relterm directly into the QK^T PSUM tile (zero
     columns contribute nothing), so no separate relterm buffer, adds,
     or reductions exist at all.

Per-core engine plan:
  PE    : q/k/v projections (fp16), q'/k' transposes, QK^T+mask (one
          k=65 matmul), 64 rel pair-matmuls per row tile, w transposes,
          AV, Wo partial.
  ACT   : PSUM->SBUF copies (with f32->fp16 casts), exp with fused
          row-sum.
  DVE   : RoPE, block-diag stationary strided writes, softmax
          max/normalize.
  DMA   : rel shard streamed as 32 x 1MB tiles, fully overlapped.
"""

import sys

sys.path.insert(0, "/opt/trn_rl_repo")

from contextlib import ExitStack

import numpy as np

import concourse.bass as bass
import concourse.tile as tile
from concourse import mybir
from concourse.masks import make_identity

# problem dims (hardcoded per spec)
B, N, DIM, H, D = 1, 512, 512, 8, 64
INNER = H * D
N_CORES = 8
P = 128                 # SBUF partitions
NT = N // P             # 4 row tiles
KT = DIM // P           # 4 contraction tiles
NPAIR = P // 2          # 64 row pairs per row tile
G = 16                  # pairs per rel DMA (128p x 16KB = 2MB)
NG = NPAIR // G         # 4 DMA groups per row tile
FP8 = True              # stream rel as fp8 e4m3 + DoubleRow quad matmuls
NQ = P // 4             # 32 row quads per row tile (fp8 path)
GQ = 16                 # quads per rel DMA (128p x 16KB = 2MB)
NGQ = NQ // GQ          # 2 DMA groups per row tile (fp8 path)
REL_AMP = 64.0          # host premultiplier on rel (fp8 range use)
Q_AMP = 8.0             # on-device premultiplier on q' diagonals (fp8)
AMP = REL_AMP * Q_AMP   # net scale on dots, undone in the exp
SCALE = D ** -0.5
MASK_BIG = 60000.0      # fp16-safe additive mask magnitude

f32 = mybir.dt.float32
f16 = mybir.dt.float16
f8 = mybir.dt.float8e4
u8 = mybir.dt.uint8
AX = mybir.AxisListType
ALU = mybir.AluOpType
AF = mybir.ActivationFunctionType


def legalize_multi_waits(nc):
    """This walrus build supports only one sync-wait per instruction; hoist
    extra waits onto same-engine NoOps placed immediately before."""
    nid = 0
    for fn in nc.m.functions:
        for bb in fn.blocks:
            new = []
            changed = False
            for inst in bb.instructions:
                si = inst.sync_info
                waits = si.on_wait if si is not None else []
                if len(waits) > 1:
                    for w in waits[:-1]:
                        nop = mybir.InstNoOp(name=f"I-waitfix-{nid}")
                        nid += 1
                        nop.engine = inst.engine
                        nop.sync_info = mybir.SyncInfo(on_wait=[w], on_update=[])
                        new.append(nop)
                    si.on_wait = [waits[-1]]
                    inst.sync_info = si
                    changed = True
                new.append(inst)
            if changed:
                bb.instructions = new


def build_nc():
    nc = bass.Bass()

    xT_ext = nc.declare_dram_parameter("xT", [DIM, N], f16, isOutput=False)
    wqkv_ext = nc.declare_dram_parameter("wqkv", [DIM, 3 * D], f16, isOutput=False)
    bqkv_ext = nc.declare_dram_parameter("bqkv", [1, 3 * D], f16, isOutput=False)
    wo_ext = nc.declare_dram_parameter("wo", [D, DIM], f16, isOutput=False)
    rope_ext = nc.declare_dram_parameter("rope", [N, D], f32, isOutput=False)
    mask_ext = nc.declare_dram_parameter("mask", [1, N], u8, isOutput=False)
    if FP8:
        rel_ext = nc.declare_dram_parameter("rel", [NT, NGQ, P, GQ, 2, N], f8,
                                            isOutput=False)
    else:
        rel_ext = nc.declare_dram_parameter("rel", [NT, NG, P, G, N], f16,
                                            isOutput=False)
    out_ext = nc.declare_dram_parameter("out", [N, DIM], f16, isOutput=True)

    with tile.TileContext(nc) as tc, ExitStack() as ctx:
        dma = nc.sync      # HWDGE; inputs first, then the rel_pos stream
        dma2 = nc.scalar   # HWDGE; outputs (kept off the rel stream ring)
        consts = ctx.enter_context(tc.tile_pool(name="consts", bufs=1))
        # PSUM: 8 banks -- 2 transpose + 2 dots + 1 av + 2 out/proj
        psum_t = ctx.enter_context(
            tc.tile_pool(name="psum_t", bufs=2, space=bass.MemorySpace.PSUM))
        psum_mm = ctx.enter_context(
            tc.tile_pool(name="psum_mm", bufs=2, space=bass.MemorySpace.PSUM))
        psum_av = ctx.enter_context(
            tc.tile_pool(name="psum_av", bufs=1, space=bass.MemorySpace.PSUM))
        psum_o = ctx.enter_context(
            tc.tile_pool(name="psum_o", bufs=1, space=bass.MemorySpace.PSUM))
        psum_p = ctx.enter_context(
            tc.tile_pool(name="psum_p", bufs=2, space=bass.MemorySpace.PSUM))
        pro = ctx.enter_context(tc.tile_pool(name="pro", bufs=1))
        relp = ctx.enter_context(tc.tile_pool(name="relp", bufs=6))
        sm = ctx.enter_context(tc.tile_pool(name="sm", bufs=2))
        outp = ctx.enter_context(tc.tile_pool(name="outp", bufs=2))

        # ---- constants ----
        # block-diagonal stationaries: qbd[k, it, pair, col]; zeros persist,
        # only the two diagonals are rewritten per row tile. Tile 0's zeros
        # are the first DVE op (critical path); tiles 1-3 are zeroed inside
        # the prologue loop so they overlap the rel stream.
        if FP8:
            qbd = consts.tile([P, NT, NQ, 2, P], f8)
            qbd_z = qbd.rearrange("p t a b c -> p t (a b c)")
        else:
            qbd = consts.tile([P, NT, NPAIR, P], f16)
            qbd_z = qbd.rearrange("p t a b -> p t (a b)")
        # zero-fill the stationary slots up front on ACT via the uint32
        # bitcast (MEMSET runs at 1x on DVE; this is ~7x cheaper and keeps
        # DVE/GpSimd free for the diagonal writes and stream gates)
        for t in range(NT):
            nc.scalar.memzero(qbd_z[:, t, :])
        ones_col = consts.tile([1, P], f16)
        nc.vector.memset(ones_col, 1.0)
        ident16 = consts.tile([P, P], f16)
        make_identity(nc, ident16[:])

        # ---- load small inputs (scalar ring, proj inputs first) ----
        x_sb = pro.tile([P, KT, N], f16)         # xT: partition = k % 128
        dma.dma_start(out=x_sb[:], in_=xT_ext.rearrange("(u p) n -> p u n", p=P))
        w_sb = pro.tile([P, KT, 3 * D], f16)
        dma2.dma_start(out=w_sb[:], in_=wqkv_ext.rearrange("(u p) n -> p u n", p=P))
        b_sb = pro.tile([1, 3 * D], f16)
        dma2.dma_start(out=b_sb[:], in_=bqkv_ext[:])
        rope_sb = pro.tile([P, NT, D], f32)
        dma2.dma_start(out=rope_sb[:], in_=rope_ext.rearrange("(t p) d -> p t d", p=P))
        wo_sb = consts.tile([D, DIM], f16)
        dma2.dma_start(out=wo_sb[:], in_=wo_ext[:])

        # q'^T / k'^T with an extra row 64: ones (q side) x additive mask row
        # (k side) so one k=65 matmul computes QK^T + mask bias.
        qpT16 = consts.tile([D + 1, N], f16)
        kpT16 = consts.tile([D + 1, N], f16)
        nc.vector.memset(qpT16[D:D + 1, :], 1.0)
        masku8 = pro.tile([D + 1, N], u8)
        dma2.dma_start(out=masku8[D:D + 1, :], in_=mask_ext[:])
        maskf = pro.tile([D + 1, N], f32)
        nc.vector.tensor_copy(maskf[D:D + 1, :], masku8[D:D + 1, :])
        nc.vector.tensor_scalar(kpT16[D:D + 1, :], maskf[D:D + 1, :],
                                MASK_BIG, -MASK_BIG, ALU.mult, ALU.add)

        # ---- cos/sin (ACT): cos(x) = sin(x + pi/2); fold QK scale into k's ----
        sin_sb = pro.tile([P, NT, D], f32)
        cos_sb = pro.tile([P, NT, D], f32)
        halfpi = consts.tile([P, 1], f32)
        nc.vector.memset(halfpi, float(np.pi / 2))
        nc.scalar.activation(sin_sb[:], rope_sb[:], AF.Sin)
        nc.scalar.activation(cos_sb[:], rope_sb[:], AF.Sin, bias=halfpi[:])
        kscale = SCALE * (AMP if FP8 else 1.0)
        sink_sb = pro.tile([P, NT, D], f32)
        cosk_sb = pro.tile([P, NT, D], f32)
        nc.vector.tensor_scalar_mul(sink_sb[:], sin_sb[:], kscale)
        nc.vector.tensor_scalar_mul(cosk_sb[:], cos_sb[:], kscale)

        # ---- per-row-tile prologue: q/k/v proj -> RoPE -> transposes ----
        q_sb = pro.tile([P, NT, D], f32)
        k_sb = pro.tile([P, NT, D], f32)
        v_sb = consts.tile([P, NT, D], f16)
        qp_sb = pro.tile([P, NT, D], f16)
        kp_sb = pro.tile([P, NT, D], f16)
        qph_sb = consts.tile([P, NT, D], f16)   # rows 64:128: q'^T cols 0:64
        qbd_flat = qbd.rearrange("p t a b c -> p (t a b c)" if FP8 else
                                 "p t a b -> p (t a b)")
        for t in range(NT):
            ps = psum_p.tile([P, 3 * D], f32, tag="proj")
            for u in range(KT):
                nc.tensor.matmul(ps[:], x_sb[:, u, t * P:(t + 1) * P],
                                 w_sb[:, u, :], start=(u == 0), stop=False)
            nc.tensor.matmul(ps[:], ones_col[:, 0:P], b_sb[:],
                             start=False, stop=True)
            nc.scalar.copy(q_sb[:, t, :], ps[:, 0:D])
            nc.scalar.copy(k_sb[:, t, :], ps[:, D:2 * D])
            nc.scalar.copy(v_sb[:, t, :], ps[:, 2 * D:3 * D])
            for (src, dst, c, s) in ((q_sb, qp_sb, cos_sb, sin_sb),
                                     (k_sb, kp_sb, cosk_sb, sink_sb)):
                sr = src[:, t, :].rearrange("p (m two) -> p m two", two=2)
                rot = pro.tile([P, D // 2, 2], f32, tag="rot")
                nc.vector.tensor_scalar_mul(rot[:, :, 0], sr[:, :, 1], -1.0)
                nc.vector.tensor_copy(rot[:, :, 1], sr[:, :, 0])
                tmp = pro.tile([P, D], f32, tag="ropetmp")
                nc.vector.tensor_mul(tmp[:], rot.rearrange("p m two -> p (m two)"),
                                     s[:, t, :])
                nc.vector.tensor_mul(dst[:, t, :], src[:, t, :], c[:, t, :])
                nc.vector.tensor_add(dst[:, t, :], dst[:, t, :], tmp[:])
            # full q' transpose (64, 128) at psum base 0 (fp16)
            ps1 = psum_t.tile([P, P], f16, tag="tp")
            nc.tensor.transpose(ps1[0:D, :], qp_sb[:, t, :], ident16[:])
            nc.scalar.copy(qpT16[0:D, t * P:(t + 1) * P], ps1[0:D, :])
            # q'^T cols 0:64 shifted to partitions 64:128 for the hi-half
            # diagonals: tiny SBUF->SBUF DMA on the SWDGE ring (transpose
            # matmuls cannot write psum partition 64)
            nc.gpsimd.dma_start(out=qph_sb[D:P, t, :],
                                in_=qpT16[0:D, t * P:t * P + D])
            if FP8:
                # quad-stationary diagonals (x Q_AMP, fp16->fp8 strided):
                # quad qd covers i_locals {qd+64, qd, qd+96, qd+32} at
                # (ks0 k-lo, ks0 k-hi, ks1 k-lo, ks1 k-hi), cols = i_local
                base = t * NQ * 2 * P
                st = 2 * P + 1
                top = base + NQ * 2 * P
                tp0 = t * P
                for off, dst_lo, s0, s1 in ((D, True, D, D + NQ),
                                            (0, False, 0, NQ),
                                            (3 * D + NQ, True, 3 * NQ, P),
                                            (2 * D + NQ, False, NQ, 2 * NQ)):
                    if dst_lo:
                        nc.vector.tensor_scalar_mul(
                            qbd_flat[0:D, base + off:top:st],
                            qpT16[0:D, tp0 + s0:tp0 + s1], Q_AMP)
                    else:
                        nc.vector.tensor_scalar_mul(
                            qbd_flat[D:P, base + off:top:st],
                            qph_sb[D:P, t, s0:s1], Q_AMP)
            else:
                # pair-stationary diagonals (f32->f16 strided):
                # col p+64 (k 0:64) <- q'[base+64+p]; col p (k 64:128) <- q'[base+p]
                base = t * NPAIR * P
                nc.vector.tensor_copy(
                    qbd_flat[0:D, base + D:base + NPAIR * P:P + 1],
                    ps1[0:D, D:P])
                nc.vector.tensor_copy(
                    qbd_flat[D:P, base:base + NPAIR * P:P + 1],
                    ps2[D:P, 0:D])
            # k' transpose (fp16)
            ps3 = psum_t.tile([P, P], f16, tag="tp")
            nc.tensor.transpose(ps3[0:D, :], kp_sb[:, t, :], ident16[:])
            nc.scalar.copy(kpT16[0:D, t * P:(t + 1) * P], ps3[0:D, :])

        # ---- main loop over row tiles ----
        def softmax_av_out(it, dots_ps):
            unscale = 1.0 / AMP if FP8 else 1.0
            # no row-max pass: |dots| <= ~10 so exp() is f32-safe, and masked
            # lanes sit at <= -100 where exp underflows to 0 as required
            w_sm = sm.tile([P, N], f16, tag="w_sm")
            rowsum = sm.tile([P, 1], f32, tag="rowsum")
            nc.scalar.activation(w_sm[:], dots_ps[:], AF.Exp, scale=unscale,
                                 accum_out=rowsum[:])
            rcp = sm.tile([P, 1], f32, tag="rcp")
            nc.vector.reciprocal(rcp[:], rowsum[:])
            wT16 = outp.tile([P, NT, P], f16, tag="wT16")
            for jt in range(NT):
                wT_ps = psum_t.tile([P, P], f16, tag="tp")
                nc.tensor.transpose(wT_ps[:], w_sm[:, jt * P:(jt + 1) * P],
                                    ident16[:])
                nc.scalar.copy(wT16[:, jt, :], wT_ps[:])
            attn_ps = psum_av.tile([D, P], f32, tag="attn")
            for jt in range(NT):
                nc.tensor.matmul(attn_ps[:], v_sb[:, jt, :], wT16[:, jt, :],
                                 start=(jt == 0), stop=(jt == NT - 1))
            attn16 = outp.tile([D, P], f16, tag="attn16")
            nc.scalar.copy(attn16[:], attn_ps[:])
            out_ps = psum_o.tile([P, DIM], f32, tag="out_ps")
            nc.tensor.matmul(out_ps[:], attn16[:], wo_sb[:], start=True, stop=True)
            # softmax normalization folded in here: rows scale by 1/rowsum
            o_sb = outp.tile([P, DIM], f16, tag="o_sb")
            nc.scalar.mul(o_sb[:], out_ps[:], rcp[:])
            dma2.dma_start(out=out_ext[it * P:(it + 1) * P, :], in_=o_sb[:])

        # ring A (sync) carries x then even rel groups; ring B (scalar) the
        # other inputs then odd rel groups -- gate the first 6 in-flight
        # groups on each ring's last input so no rel DMA overtakes them
        gate_a = x_sb[0:1, 0, 0:1]
        gate_b = masku8[D:D + 1, 0:1]
        NGROUP = NGQ if FP8 else NG
        for it in range(NT):
            dots_ps = psum_mm.tile([P, N], f32, tag="mm")
            # rel matmuls first (start=True on the first clears the bank),
            # QK^T + mask joins last so it never gates the stream.
            for gr in range(NGROUP):
                if FP8:
                    rl = relp.tile([P, GQ, 2, N], f8)
                else:
                    rl = relp.tile([P, G, N], f16)
                gidx = it * NGROUP + gr
                if gidx < 6:
                    nc.gpsimd.tensor_copy(
                        rl.rearrange("p a b -> p (a b)" if not FP8 else
                                     "p a b c -> p (a b c)")[0:1, 0:1],
                        gate_a if gidx % 2 == 0 else gate_b)
                ring = dma if gidx % 2 == 0 else dma2
                ring.dma_start(out=rl[:], in_=rel_ext[it, gr])
                if FP8:
                    for g8 in range(GQ):
                        qd = gr * GQ + g8
                        nc.tensor.matmul(dots_ps[:], qbd[:, it, qd, :, :],
                                         rl[:, g8, :, :], start=(qd == 0),
                                         stop=False,
                                         perf_mode=mybir.MatmulPerfMode.DoubleRow)
                else:
                    for g8 in range(G):
                        pr = gr * G + g8
                        nc.tensor.matmul(dots_ps[:], qbd[:, it, pr, :],
                                         rl[:, g8, :], start=(pr == 0),
                                         stop=False)
            nc.tensor.matmul(dots_ps[:], qpT16[:, it * P:(it + 1) * P], kpT16[:],
                             start=False, stop=True)
            softmax_av_out(it, dots_ps)

    legalize_multi_waits(nc)
    return nc


_NC_CACHE = None
TRACE = False        # set by test harness to capture an NTFF profile
LAST_RESULT = None   # BassKernelResults of the most recent kernel() call


def _get_nc():
    global _NC_CACHE
    if _NC_CACHE is None:
        _NC_CACHE = build_nc()
    return _NC_CACHE


def _repack_rel(rel_h):
    """(N, N, D) f32 -> (NT, NG, 2*D, G, N) fp16 pair layout: each (it, gr)
    DMA group is one fully contiguous DRAM block, partition k major inside;
    k rows 0:64 hold d of i1=base+64+p (odd group), 64:128 of i0=base+p."""
    r = rel_h.transpose(0, 2, 1)                       # (i, d, j)
    rh = r.reshape(NT, 2, NPAIR, D, N)                 # (it, g, p, d, j)
    a = rh[:, ::-1].transpose(0, 1, 3, 2, 4).reshape(NT, P, NPAIR, N)
    a = a.reshape(NT, P, NG, G, N).transpose(0, 2, 1, 3, 4)
    return np.ascontiguousarray(a, dtype=np.float16)


def _repack_rel_fp8(rel_h):
    """(N, N, D) f32 -> (NT, NGQ, 2*D, GQ, 2, N) e4m3 quad layout (x REL_AMP):
    quad q covers i_locals {q+64, q, q+96, q+32} on (ks, k-half) slots
    (0,lo), (0,hi), (1,lo), (1,hi); each (it, gr) block is contiguous."""
    r = rel_h.transpose(0, 2, 1)                       # (i, d, j)
    b5 = r.reshape(NT, 4, NQ, D, N)                    # blk = i_local // 32
    b6 = b5[:, [2, 0, 3, 1]]                           # s = 2*ks + half
    a = b6.reshape(NT, 2, 2, NQ, D, N).transpose(0, 2, 4, 3, 1, 5)
    a = a.reshape(NT, P, NQ, 2, N)
    a = a.reshape(NT, P, NGQ, GQ, 2, N).transpose(0, 2, 1, 3, 4, 5)
    import ml_dtypes
    return np.ascontiguousarray((a * REL_AMP).astype(ml_dtypes.float8_e4m3))


def kernel(**inputs):
    x = np.asarray(inputs["x"], dtype=np.float32)
    mask = np.asarray(inputs["mask"])
    rope = np.asarray(inputs["rope"], dtype=np.float32)
    rel_pos = np.asarray(inputs["rel_pos"], dtype=np.float32)
    Wq = np.asarray(inputs["Wq"], dtype=np.float32)
    bq = np.asarray(inputs["bq"], dtype=np.float32)
    Wk = np.asarray(inputs["Wk"], dtype=np.float32)
    bk = np.asarray(inputs["bk"], dtype=np.float32)
    Wv = np.asarray(inputs["Wv"], dtype=np.float32)
    bv = np.asarray(inputs["bv"], dtype=np.float32)
    Wo = np.asarray(inputs["Wo"], dtype=np.float32)
    bo = np.asarray(inputs["bo"], dtype=np.float32)

    nc = _get_nc()

    xT = np.ascontiguousarray(x.reshape(N, DIM).T).astype(np.float16)
    mask_u8 = np.ascontiguousarray(mask.reshape(1, N).astype(np.uint8, copy=False))
    rope2 = np.ascontiguousarray(rope)

    in_maps = []
    for h in range(N_CORES):
        sl = slice(h * D, (h + 1) * D)
        wqkv = np.concatenate([Wq[:, sl], Wk[:, sl], Wv[:, sl]],
                              axis=1).astype(np.float16)
        bqkv = np.concatenate([bq[sl], bk[sl], bv[sl]])[None, :].astype(np.float16)
        in_maps.append({
            "xT": xT,
            "wqkv": np.ascontiguousarray(wqkv),
            "bqkv": np.ascontiguousarray(bqkv),
            "wo": np.ascontiguousarray(Wo[sl, :]).astype(np.float16),
            "rope": rope2,
            "mask": mask_u8,
            "rel": (_repack_rel_fp8 if FP8 else _repack_rel)(rel_pos[0, h]),
        })

    from concourse.bass_utils import run_bass_kernel_spmd
    res = run_bass_kernel_spmd(nc, in_maps, list(range(N_CORES)), trace=TRACE)
    globals()["LAST_RESULT"] = res
    out = np.zeros((N, DIM), dtype=np.float32)
    for h in range(N_CORES):
        out += res.results[h]["out"].astype(np.float32)
    out += bo[None, :]
    return out.reshape(B, N, DIM)


# revision 34
# speedup vs baseline: 1.0399x; 1.0399x over previous
"""Bass/Trainium2 kernel for nn_Attention_66297115181568 (sparse_attention).

Strategy: head-parallel across 8 NeuronCores. Core h computes head h
end-to-end; the host sums the 8 partial (512, 512) outputs (the
tensor-parallel all-reduce done at unshard time) and adds bo.

The dominant cost is streaming rel_pos (67MB/core in f32). Two changes
vs the v1 kernel:
  1. rel_pos is cast to fp16 and transposed to (i-tile, pair, k, j)
     layout ON THE HOST: DMA bytes halve to 33.5MB/core (~94us at the
     ~356GB/s per-core HBM ceiling).
  2. The rel contraction relterm[i,j] = sum_d q'[i,d]*rel[i,j,d] moves
     from DVE/ACT/GpSimd onto the idle tensor engine: for each pair of
     rows (i0=base+p, i1=base+64+p) a 128x128 stationary holds q'[i0]
     in column p (k rows 64:128) and q'[i1] in column p+64 (k rows
     0:64), zeros elsewhere; the moving operand is the (128, 512) rel
     tile with both rows' d-vectors stacked on partitions. 64 such
     matmuls accumulate relterm directly into the QK^T PSUM tile (zero
     columns contribute nothing), so no separate relterm buffer, adds,
     or reductions exist at all.

Per-core engine plan:
  PE    : q/k/v projections (fp16), q'/k' transposes, QK^T+mask (one
          k=65 matmul), 64 rel pair-matmuls per row tile, w transposes,
          AV, Wo partial.
  ACT   : PSUM->SBUF copies (with f32->fp16 casts), exp with fused
          row-sum.
  DVE   : RoPE, block-diag stationary strided writes, softmax
          max/normalize.
  DMA   : rel shard streamed as 32 x 1MB tiles, fully overlapped.
"""

import sys

sys.path.insert(0, "/opt/trn_rl_repo")

from contextlib import ExitStack

import numpy as np

import concourse.bass as bass
import concourse.tile as tile
from concourse import mybir
from concourse.masks import make_identity

# problem dims (hardcoded per spec)
B, N, DIM, H, D = 1, 512, 512, 8, 64
INNER = H * D
N_CORES = 8
P = 128                 # SBUF partitions
NT = N // P             # 4 row tiles
KT = DIM // P           # 4 contraction tiles
NPAIR = P // 2          # 64 row pairs per row tile
G = 16                  # pairs per rel DMA (128p x 16KB = 2MB)
NG = NPAIR // G         # 4 DMA groups per row tile
FP8 = True              # stream rel as fp8 e4m3 + DoubleRow quad matmuls
NQ = P // 4             # 32 row quads per row tile (fp8 path)
GQ = 16                 # quads per rel DMA (128p x 16KB = 2MB)
NGQ = NQ // GQ          # 2 DMA groups per row tile (fp8 path)
REL_AMP = 64.0          # host premultiplier on rel (fp8 range use)
Q_AMP = 8.0             # on-device premultiplier on q' diagonals (fp8)
AMP = REL_AMP * Q_AMP   # net scale on dots, undone in the exp
SCALE = D ** -0.5
MASK_BIG = 60000.0      # fp16-safe additive mask magnitude

f32 = mybir.dt.float32
f16 = mybir.dt.float16
f8 = mybir.dt.float8e4
u8 = mybir.dt.uint8
AX = mybir.AxisListType
ALU = mybir.AluOpType
AF = mybir.ActivationFunctionType


def legalize_multi_waits(nc):
    """This walrus build supports only one sync-wait per instruction; hoist
    extra waits onto same-engine NoOps placed immediately before."""
    nid = 0
    for fn in nc.m.functions:
        for bb in fn.blocks:
            new = []
            changed = False
            for inst in bb.instructions:
                si = inst.sync_info
                waits = si.on_wait if si is not None else []
                if len(waits) > 1:
                    for w in waits[:-1]:
                        nop = mybir.InstNoOp(name=f"I-waitfix-{nid}")
                        nid += 1
                        nop.engine = inst.engine
                        nop.sync_info = mybir.SyncInfo(on_wait=[w], on_update=[])
                        new.append(nop)
                    si.on_wait = [waits[-1]]
                    inst.sync_info = si
                    changed = True
                new.append(inst)
            if changed:
                bb.instructions = new


def build_nc():
    nc = bass.Bass()

    xT_ext = nc.declare_dram_parameter("xT", [DIM, N], f16, isOutput=False)
    wqkv_ext = nc.declare_dram_parameter("wqkv", [DIM, 3 * D], f16, isOutput=False)
    bqkv_ext = nc.declare_dram_parameter("bqkv", [1, 3 * D], f16, isOutput=False)
    wo_ext = nc.declare_dram_parameter("wo", [D, DIM], f16, isOutput=False)
    rope_ext = nc.declare_dram_parameter("rope", [N, D], f32, isOutput=False)
    mask_ext = nc.declare_dram_parameter("mask", [1, N], u8, isOutput=False)
    if FP8:
        rel_ext = nc.declare_dram_parameter("rel", [NT, NGQ, P, GQ, 2, N], f8,
                                            isOutput=False)
    else:
        rel_ext = nc.declare_dram_parameter("rel", [NT, NG, P, G, N], f16,
                                            isOutput=False)
    out_ext = nc.declare_dram_parameter("out", [N, DIM], f16, isOutput=True)

    with tile.TileContext(nc) as tc, ExitStack() as ctx:
        dma = nc.sync      # HWDGE; inputs first, then the rel_pos stream
        dma2 = nc.scalar   # HWDGE; outputs (kept off the rel stream ring)
        consts = ctx.enter_context(tc.tile_pool(name="consts", bufs=1))
        # PSUM: 8 banks -- 2 transpose + 2 dots + 1 av + 2 out/proj
        psum_t = ctx.enter_context(
            tc.tile_pool(name="psum_t", bufs=2, space=bass.MemorySpace.PSUM))
        psum_mm = ctx.enter_context(
            tc.tile_pool(name="psum_mm", bufs=2, space=bass.MemorySpace.PSUM))
        psum_av = ctx.enter_context(
            tc.tile_pool(name="psum_av", bufs=1, space=bass.MemorySpace.PSUM))
        psum_o = ctx.enter_context(
            tc.tile_pool(name="psum_o", bufs=1, space=bass.MemorySpace.PSUM))
        psum_p = ctx.enter_context(
            tc.tile_pool(name="psum_p", bufs=2, space=bass.MemorySpace.PSUM))
        pro = ctx.enter_context(tc.tile_pool(name="pro", bufs=1))
        relp = ctx.enter_context(tc.tile_pool(name="relp", bufs=6))
        sm = ctx.enter_context(tc.tile_pool(name="sm", bufs=2))
        outp = ctx.enter_context(tc.tile_pool(name="outp", bufs=2))

        # ---- constants ----
        # block-diagonal stationaries: qbd[k, it, pair, col]; zeros persist,
        # only the two diagonals are rewritten per row tile. Tile 0's zeros
        # are the first DVE op (critical path); tiles 1-3 are zeroed inside
        # the prologue loop so they overlap the rel stream.
        if FP8:
            qbd = consts.tile([P, NT, NQ, 2, P], f8)
            qbd_z = qbd.rearrange("p t a b c -> p t (a b c)")
        else:
            qbd = consts.tile([P, NT, NPAIR, P], f16)
            qbd_z = qbd.rearrange("p t a b -> p t (a b)")
        # zero-fill the stationary slots up front on ACT via the uint32
        # bitcast (MEMSET runs at 1x on DVE; this is ~7x cheaper and keeps
        # DVE/GpSimd free for the diagonal writes and stream gates)
        for t in range(NT):
            nc.scalar.memzero(qbd_z[:, t, :])
        ident = consts.tile([P, P], f32)
        make_identity(nc, ident[:])
        ones_col = consts.tile([1, P], f16)
        nc.vector.memset(ones_col, 1.0)
        ident16 = consts.tile([P, P], f16)
        nc.vector.tensor_copy(ident16[:], ident[:])

        # ---- load small inputs (scalar ring, proj inputs first) ----
        x_sb = pro.tile([P, KT, N], f16)         # xT: partition = k % 128
        dma.dma_start(out=x_sb[:], in_=xT_ext.rearrange("(u p) n -> p u n", p=P))
        w_sb = pro.tile([P, KT, 3 * D], f16)
        dma2.dma_start(out=w_sb[:], in_=wqkv_ext.rearrange("(u p) n -> p u n", p=P))
        b_sb = pro.tile([1, 3 * D], f16)
        dma2.dma_start(out=b_sb[:], in_=bqkv_ext[:])
        rope_sb = pro.tile([P, NT, D], f32)
        dma2.dma_start(out=rope_sb[:], in_=rope_ext.rearrange("(t p) d -> p t d", p=P))
        wo_sb = consts.tile([D, DIM], f16)
        dma2.dma_start(out=wo_sb[:], in_=wo_ext[:])

        # q'^T / k'^T with an extra row 64: ones (q side) x additive mask row
        # (k side) so one k=65 matmul computes QK^T + mask bias.
        qpT16 = consts.tile([D + 1, N], f16)
        kpT16 = consts.tile([D + 1, N], f16)
        nc.vector.memset(qpT16[D:D + 1, :], 1.0)
        masku8 = pro.tile([D + 1, N], u8)
        dma2.dma_start(out=masku8[D:D + 1, :], in_=mask_ext[:])
        maskf = pro.tile([D + 1, N], f32)
        nc.vector.tensor_copy(maskf[D:D + 1, :], masku8[D:D + 1, :])
        nc.vector.tensor_scalar(kpT16[D:D + 1, :], maskf[D:D + 1, :],
                                MASK_BIG, -MASK_BIG, ALU.mult, ALU.add)

        # ---- cos/sin (ACT): cos(x) = sin(x + pi/2); fold QK scale into k's ----
        sin_sb = pro.tile([P, NT, D], f32)
        cos_sb = pro.tile([P, NT, D], f32)
        halfpi = consts.tile([P, 1], f32)
        nc.vector.memset(halfpi, float(np.pi / 2))
        nc.scalar.activation(sin_sb[:], rope_sb[:], AF.Sin)
        nc.scalar.activation(cos_sb[:], rope_sb[:], AF.Sin, bias=halfpi[:])
        kscale = SCALE * (AMP if FP8 else 1.0)
        sink_sb = pro.tile([P, NT, D], f32)
        cosk_sb = pro.tile([P, NT, D], f32)
        nc.vector.tensor_scalar_mul(sink_sb[:], sin_sb[:], kscale)
        nc.vector.tensor_scalar_mul(cosk_sb[:], cos_sb[:], kscale)

        # ---- per-row-tile prologue: q/k/v proj -> RoPE -> transposes ----
        q_sb = pro.tile([P, NT, D], f32)
        k_sb = pro.tile([P, NT, D], f32)
        v_sb = consts.tile([P, NT, D], f16)
        qp_sb = pro.tile([P, NT, D], f32)
        kp_sb = pro.tile([P, NT, D], f32)
        qbd_flat = qbd.rearrange("p t a b c -> p (t a b c)" if FP8 else
                                 "p t a b -> p (t a b)")
        for t in range(NT):
            ps = psum_p.tile([P, 3 * D], f32, tag="proj")
            for u in range(KT):
                nc.tensor.matmul(ps[:], x_sb[:, u, t * P:(t + 1) * P],
                                 w_sb[:, u, :], start=(u == 0), stop=False)
            nc.tensor.matmul(ps[:], ones_col[:, 0:P], b_sb[:],
                             start=False, stop=True)
            nc.scalar.copy(q_sb[:, t, :], ps[:, 0:D])
            nc.scalar.copy(k_sb[:, t, :], ps[:, D:2 * D])
            nc.scalar.copy(v_sb[:, t, :], ps[:, 2 * D:3 * D])
            for (src, dst, c, s) in ((q_sb, qp_sb, cos_sb, sin_sb),
                                     (k_sb, kp_sb, cosk_sb, sink_sb)):
                sr = src[:, t, :].rearrange("p (m two) -> p m two", two=2)
                rot = pro.tile([P, D // 2, 2], f32, tag="rot")
                nc.vector.tensor_scalar_mul(rot[:, :, 0], sr[:, :, 1], -1.0)
                nc.vector.tensor_copy(rot[:, :, 1], sr[:, :, 0])
                tmp = pro.tile([P, D], f32, tag="ropetmp")
                nc.vector.tensor_mul(tmp[:], rot.rearrange("p m two -> p (m two)"),
                                     s[:, t, :])
                nc.vector.tensor_mul(dst[:, t, :], src[:, t, :], c[:, t, :])
                nc.vector.tensor_add(dst[:, t, :], dst[:, t, :], tmp[:])
            # full q' transpose (64, 128) at psum base 0
            ps1 = psum_t.tile([P, P], f32, tag="tp")
            nc.tensor.transpose(ps1[0:D, :], qp_sb[:, t, :], ident[:])
            nc.scalar.copy(qpT16[0:D, t * P:(t + 1) * P], ps1[0:D, :])
            # half transpose of q' rows 0:64 (even group) at psum base 64.
            # Transpose-mode matmuls must write psum partition 0, so use a
            # regular matmul (qp16 as stationary x identity) col-tiled to 64.
            qp16 = pro.tile([D, D], f16, tag="qp16")
            nc.vector.tensor_copy(qp16[:], qp_sb[0:D, t, :])
            ps2 = psum_t.tile([P, P], f32, tag="tp")
            nc.tensor.matmul(ps2[D:P, 0:D], qp16[:], ident16[0:D, 0:D],
                             start=True, stop=True)
            if FP8:
                # quad-stationary diagonals (x Q_AMP, f32->fp8 strided):
                # quad qd covers i_locals {qd+64, qd, qd+96, qd+32} at
                # (ks0 k-lo, ks0 k-hi, ks1 k-lo, ks1 k-hi), cols = i_local
                base = t * NQ * 2 * P
                st = 2 * P + 1
                top = base + NQ * 2 * P
                for off, dst_lo, s0, s1 in ((D, True, D, D + NQ),
                                            (0, False, 0, NQ),
                                            (3 * D + NQ, True, 3 * NQ, P),
                                            (2 * D + NQ, False, NQ, 2 * NQ)):
                    srcp = ps1 if dst_lo else ps2
                    dst = (qbd_flat[0:D, base + off:top:st] if dst_lo else
                           qbd_flat[D:P, base + off:top:st])
                    nc.vector.tensor_scalar_mul(
                        dst, (srcp[0:D, s0:s1] if dst_lo else
                              srcp[D:P, s0:s1]), Q_AMP)
            else:
                # pair-stationary diagonals (f32->f16 strided):
                # col p+64 (k 0:64) <- q'[base+64+p]; col p (k 64:128) <- q'[base+p]
                base = t * NPAIR * P
                nc.vector.tensor_copy(
                    qbd_flat[0:D, base + D:base + NPAIR * P:P + 1],
                    ps1[0:D, D:P])
                nc.vector.tensor_copy(
                    qbd_flat[D:P, base:base + NPAIR * P:P + 1],
                    ps2[D:P, 0:D])
            # k' transpose
            ps3 = psum_t.tile([P, P], f32, tag="tp")
            nc.tensor.transpose(ps3[0:D, :], kp_sb[:, t, :], ident[:])
            nc.scalar.copy(kpT16[0:D, t * P:(t + 1) * P], ps3[0:D, :])

        # ---- main loop over row tiles ----
        def softmax_av_out(it, dots_ps):
            unscale = 1.0 / AMP if FP8 else 1.0
            # no row-max pass: |dots| <= ~10 so exp() is f32-safe, and masked
            # lanes sit at <= -100 where exp underflows to 0 as required
            w_sm = sm.tile([P, N], f32, tag="w_sm")
            rowsum = sm.tile([P, 1], f32, tag="rowsum")
            nc.scalar.activation(w_sm[:], dots_ps[:], AF.Exp, scale=unscale,
                                 accum_out=rowsum[:])
            rcp = sm.tile([P, 1], f32, tag="rcp")
            nc.vector.reciprocal(rcp[:], rowsum[:])
            wT16 = outp.tile([P, NT, P], f16, tag="wT16")
            for jt in range(NT):
                wT_ps = psum_t.tile([P, P], f32, tag="tp")
                nc.tensor.transpose(wT_ps[:], w_sm[:, jt * P:(jt + 1) * P],
                                    ident[:])
                nc.scalar.copy(wT16[:, jt, :], wT_ps[:])
            attn_ps = psum_av.tile([D, P], f32, tag="attn")
            for jt in range(NT):
                nc.tensor.matmul(attn_ps[:], v_sb[:, jt, :], wT16[:, jt, :],
                                 start=(jt == 0), stop=(jt == NT - 1))
            attn16 = outp.tile([D, P], f16, tag="attn16")
            nc.scalar.copy(attn16[:], attn_ps[:])
            out_ps = psum_o.tile([P, DIM], f32, tag="out_ps")
            nc.tensor.matmul(out_ps[:], attn16[:], wo_sb[:], start=True, stop=True)
            # softmax normalization folded in here: rows scale by 1/rowsum
            o_sb = outp.tile([P, DIM], f16, tag="o_sb")
            nc.scalar.mul(o_sb[:], out_ps[:], rcp[:])
            dma2.dma_start(out=out_ext[it * P:(it + 1) * P, :], in_=o_sb[:])

        # ring A (sync) carries x then even rel groups; ring B (scalar) the
        # other inputs then odd rel groups -- gate the first 6 in-flight
        # groups on each ring's last input so no rel DMA overtakes them
        gate_a = x_sb[0:1, 0, 0:1]
        gate_b = masku8[D:D + 1, 0:1]
        NGROUP = NGQ if FP8 else NG
        for it in range(NT):
            dots_ps = psum_mm.tile([P, N], f32, tag="mm")
            # rel matmuls first (start=True on the first clears the bank),
            # QK^T + mask joins last so it never gates the stream.
            for gr in range(NGROUP):
                if FP8:
                    rl = relp.tile([P, GQ, 2, N], f8)
                else:
                    rl = relp.tile([P, G, N], f16)
                gidx = it * NGROUP + gr
                if gidx < 6:
                    nc.gpsimd.tensor_copy(
                        rl.rearrange("p a b -> p (a b)" if not FP8 else
                                     "p a b c -> p (a b c)")[0:1, 0:1],
                        gate_a if gidx % 2 == 0 else gate_b)
                ring = dma if gidx % 2 == 0 else dma2
                ring.dma_start(out=rl[:], in_=rel_ext[it, gr])
                if FP8:
                    for g8 in range(GQ):
                        qd = gr * GQ + g8
                        nc.tensor.matmul(dots_ps[:], qbd[:, it, qd, :, :],
                                         rl[:, g8, :, :], start=(qd == 0),
                                         stop=False,
                                         perf_mode=mybir.MatmulPerfMode.DoubleRow)
                else:
                    for g8 in range(G):
                        pr = gr * G + g8
                        nc.tensor.matmul(dots_ps[:], qbd[:, it, pr, :],
                                         rl[:, g8, :], start=(pr == 0),
                                         stop=False)
            nc.tensor.matmul(dots_ps[:], qpT16[:, it * P:(it + 1) * P], kpT16[:],
                             start=False, stop=True)
            softmax_av_out(it, dots_ps)

    legalize_multi_waits(nc)
    return nc


_NC_CACHE = None
TRACE = False        # set by test harness to capture an NTFF profile
LAST_RESULT = None   # BassKernelResults of the most recent kernel() call


def _get_nc():
    global _NC_CACHE
    if _NC_CACHE is None:
        _NC_CACHE = build_nc()
    return _NC_CACHE


def _repack_rel(rel_h):
    """(N, N, D) f32 -> (NT, NG, 2*D, G, N) fp16 pair layout: each (it, gr)
    DMA group is one fully contiguous DRAM block, partition k major inside;
    k rows 0:64 hold d of i1=base+64+p (odd group), 64:128 of i0=base+p."""
    r = rel_h.transpose(0, 2, 1)                       # (i, d, j)
    rh = r.reshape(NT, 2, NPAIR, D, N)                 # (it, g, p, d, j)
    a = rh[:, ::-1].transpose(0, 1, 3, 2, 4).reshape(NT, P, NPAIR, N)
    a = a.reshape(NT, P, NG, G, N).transpose(0, 2, 1, 3, 4)
    return np.ascontiguousarray(a, dtype=np.float16)


def _repack_rel_fp8(rel_h):
    """(N, N, D) f32 -> (NT, NGQ, 2*D, GQ, 2, N) e4m3 quad layout (x REL_AMP):
    quad q covers i_locals {q+64, q, q+96, q+32} on (ks, k-half) slots
    (0,lo), (0,hi), (1,lo), (1,hi); each (it, gr) block is contiguous."""
    r = rel_h.transpose(0, 2, 1)                       # (i, d, j)
    b5 = r.reshape(NT, 4, NQ, D, N)                    # blk = i_local // 32
    b6 = b5[:, [2, 0, 3, 1]]                           # s = 2*ks + half
    a = b6.reshape(NT, 2, 2, NQ, D, N).transpose(0, 2, 4, 3, 1, 5)
    a = a.reshape(NT, P, NQ, 2, N)
    a = a.reshape(NT, P, NGQ, GQ, 2, N).transpose(0, 2, 1, 3, 4, 5)
    import ml_dtypes
    return np.ascontiguousarray((a * REL_AMP).astype(ml_dtypes.float8_e4m3))


def kernel(**inputs):
    x = np.asarray(inputs["x"], dtype=np.float32)
    mask = np.asarray(inputs["mask"])
    rope = np.asarray(inputs["rope"], dtype=np.float32)
    rel_pos = np.asarray(inputs["rel_pos"], dtype=np.float32)
    Wq = np.asarray(inputs["Wq"], dtype=np.float32)
    bq = np.asarray(inputs["bq"], dtype=np.float32)
    Wk = np.asarray(inputs["Wk"], dtype=np.float32)
    bk = np.asarray(inputs["bk"], dtype=np.float32)
    Wv = np.asarray(inputs["Wv"], dtype=np.float32)
    bv = np.asarray(inputs["bv"], dtype=np.float32)
    Wo = np.asarray(inputs["Wo"], dtype=np.float32)
    bo = np.asarray(inputs["bo"], dtype=np.float32)

    nc = _get_nc()

    xT = np.ascontiguousarray(x.reshape(N, DIM).T).astype(np.float16)
    mask_u8 = np.ascontiguousarray(mask.reshape(1, N).astype(np.uint8, copy=False))
    rope2 = np.ascontiguousarray(rope)

    in_maps = []
    for h in range(N_CORES):
        sl = slice(h * D, (h + 1) * D)
        wqkv = np.concatenate([Wq[:, sl], Wk[:, sl], Wv[:, sl]],
                              axis=1).astype(np.float16)
        bqkv = np.concatenate([bq[sl], bk[sl], bv[sl]])[None, :].astype(np.float16)
        in_maps.append({
            "xT": xT,
            "wqkv": np.ascontiguousarray(wqkv),
            "bqkv": np.ascontiguousarray(bqkv),
            "wo": np.ascontiguousarray(Wo[sl, :]).astype(np.float16),
            "rope": rope2,
            "mask": mask_u8,
            "rel": (_repack_rel_fp8 if FP8 else _repack_rel)(rel_pos[0, h]),
        })

    from concourse.bass_utils import run_bass_kernel_spmd
    res = run_bass_kernel_spmd(nc, in_maps, list(range(N_CORES)), trace=TRACE)
    globals()["LAST_RESULT"] = res
    out = np.zeros((N, DIM), dtype=np.float32)
    for h in range(N_CORES):
        out += res.results[h]["out"].astype(np.float32)
    out += bo[None, :]
    return out.reshape(B, N, DIM)


# revision 36
# speedup vs baseline: 1.0898x; 1.0480x over previous
"""Bass/Trainium2 kernel for nn_Attention_66297115181568 (sparse_attention).

Strategy: head-parallel across 8 NeuronCores. Core h computes head h
end-to-end; the host sums the 8 partial (512, 512) outputs (the
tensor-parallel all-reduce done at unshard time) and adds bo.

The dominant cost is streaming rel_pos (67MB/core as f32). Key ideas:
  1. rel_pos is scaled by 64, cast to fp8 e4m3, and repacked ON THE HOST
     into a quad layout: DMA bytes drop 4x to 16.8MB/core, streamed as
     2MB fully-contiguous DRAM blocks alternating across both HWDGE
     rings (~410GB/s combined).
  2. The rel contraction relterm[i,j] = sum_d q'[i,d]*rel[i,j,d] runs
     on the tensor engine as DoubleRow fp8 matmuls: for each quad of
     rows {q+64, q, q+96, q+32} a (128, 2, 128) stationary holds the
     four q' vectors (x8) on its block-diagonal, zeros elsewhere; the
     moving operand stacks the four rows' rel d-vectors on (partition,
     k-subtile). 32 such matmuls accumulate 512*relterm directly into
     the QK^T PSUM tile (zero columns add nothing) -- no separate
     relterm buffer, elementwise products, or reductions exist.
  3. QK^T (+ additive mask via an extra k=65 ones/mask row) is scaled
     by 512 to match, and one exp(x/512) on ACT undoes it; softmax
     skips the row-max pass (logits are bounded, masked lanes sit at
     <= -100 and underflow to exactly 0) and the 1/rowsum normalization
     is folded into the output copy.
  4. The block-diagonal stationary zeros are written via ACT uint32
     bitcast memzero (MEMSET runs at 1x on DVE and would gate the
     pipeline); only the 512 diagonal values are rewritten per tile by
     strided DVE casts.

Per-core engine plan:
  PE    : q/k/v projections (fp16), q'/k' transposes, QK^T+mask, 32
          DoubleRow rel matmuls per row tile, softmax-weight
          transposes, AV, Wo partial.
  ACT   : stationary memzero, PSUM->SBUF copies (f32->fp16 casts), exp
          with fused row-sum, normalized output copy.
  DVE   : RoPE, block-diagonal strided diagonal writes, reciprocal.
  GpSimd: identity build + stream-gate touches.
  DMA   : fp8 rel stream on both rings behind the small inputs, fully
          overlapped with compute.
"""

import sys

sys.path.insert(0, "/opt/trn_rl_repo")

from contextlib import ExitStack

import numpy as np

import concourse.bass as bass
import concourse.tile as tile
from concourse import mybir
from concourse.masks import make_identity

# problem dims (hardcoded per spec)
B, N, DIM, H, D = 1, 512, 512, 8, 64
INNER = H * D
N_CORES = 8
P = 128                 # SBUF partitions
NT = N // P             # 4 row tiles
KT = DIM // P           # 4 contraction tiles
NPAIR = P // 2          # 64 row pairs per row tile
G = 16                  # pairs per rel DMA (128p x 16KB = 2MB)
NG = NPAIR // G         # 4 DMA groups per row tile
FP8 = True              # stream rel as fp8 e4m3 + DoubleRow quad matmuls
NQ = P // 4             # 32 row quads per row tile (fp8 path)
GQ = 16                 # quads per rel DMA (128p x 16KB = 2MB)
NGQ = NQ // GQ          # 2 DMA groups per row tile (fp8 path)
REL_AMP = 64.0          # host premultiplier on rel (fp8 range use)
Q_AMP = 8.0             # on-device premultiplier on q' diagonals (fp8)
AMP = REL_AMP * Q_AMP   # net scale on dots, undone in the exp
SCALE = D ** -0.5
MASK_BIG = 60000.0      # fp16-safe additive mask magnitude

f32 = mybir.dt.float32
f16 = mybir.dt.float16
f8 = mybir.dt.float8e4
u8 = mybir.dt.uint8
AX = mybir.AxisListType
ALU = mybir.AluOpType
AF = mybir.ActivationFunctionType


def legalize_multi_waits(nc):
    """This walrus build supports only one sync-wait per instruction; hoist
    extra waits onto same-engine NoOps placed immediately before."""
    nid = 0
    for fn in nc.m.functions:
        for bb in fn.blocks:
            new = []
            changed = False
            for inst in bb.instructions:
                si = inst.sync_info
                waits = si.on_wait if si is not None else []
                if len(waits) > 1:
                    for w in waits[:-1]:
                        nop = mybir.InstNoOp(name=f"I-waitfix-{nid}")
                        nid += 1
                        nop.engine = inst.engine
                        nop.sync_info = mybir.SyncInfo(on_wait=[w], on_update=[])
                        new.append(nop)
                    si.on_wait = [waits[-1]]
                    inst.sync_info = si
                    changed = True
                new.append(inst)
            if changed:
                bb.instructions = new


def build_nc():
    nc = bass.Bass()

    xT_ext = nc.declare_dram_parameter("xT", [DIM, N], f16, isOutput=False)
    wqkv_ext = nc.declare_dram_parameter("wqkv", [DIM, 3 * D], f16, isOutput=False)
    bqkv_ext = nc.declare_dram_parameter("bqkv", [1, 3 * D], f16, isOutput=False)
    wo_ext = nc.declare_dram_parameter("wo", [D, DIM], f16, isOutput=False)
    rope_ext = nc.declare_dram_parameter("rope", [N, D], f32, isOutput=False)
    mask_ext = nc.declare_dram_parameter("mask", [1, N], u8, isOutput=False)
    if FP8:
        rel_ext = nc.declare_dram_parameter("rel", [NT, NGQ, P, GQ, 2, N], f8,
                                            isOutput=False)
    else:
        rel_ext = nc.declare_dram_parameter("rel", [NT, NG, P, G, N], f16,
                                            isOutput=False)
    out_ext = nc.declare_dram_parameter("out", [N, DIM], f16, isOutput=True)

    with tile.TileContext(nc) as tc, ExitStack() as ctx:
        dma = nc.sync      # HWDGE; inputs first, then the rel_pos stream
        dma2 = nc.scalar   # HWDGE; outputs (kept off the rel stream ring)
        consts = ctx.enter_context(tc.tile_pool(name="consts", bufs=1))
        # PSUM: 8 banks -- 2 transpose + 2 dots + 1 av + 2 out/proj
        psum_t = ctx.enter_context(
            tc.tile_pool(name="psum_t", bufs=2, space=bass.MemorySpace.PSUM))
        psum_mm = ctx.enter_context(
            tc.tile_pool(name="psum_mm", bufs=2, space=bass.MemorySpace.PSUM))
        psum_av = ctx.enter_context(
            tc.tile_pool(name="psum_av", bufs=1, space=bass.MemorySpace.PSUM))
        psum_o = ctx.enter_context(
            tc.tile_pool(name="psum_o", bufs=1, space=bass.MemorySpace.PSUM))
        psum_p = ctx.enter_context(
            tc.tile_pool(name="psum_p", bufs=2, space=bass.MemorySpace.PSUM))
        pro = ctx.enter_context(tc.tile_pool(name="pro", bufs=1))
        relp = ctx.enter_context(tc.tile_pool(name="relp", bufs=8))
        sm = ctx.enter_context(tc.tile_pool(name="sm", bufs=2))
        outp = ctx.enter_context(tc.tile_pool(name="outp", bufs=2))

        # ---- constants ----
        # block-diagonal stationaries: qbd[k, it, pair, col]; zeros persist,
        # only the two diagonals are rewritten per row tile. Tile 0's zeros
        # are the first DVE op (critical path); tiles 1-3 are zeroed inside
        # the prologue loop so they overlap the rel stream.
        if FP8:
            qbd = consts.tile([P, NT, NQ, 2, P], f8)
            qbd_z = qbd.rearrange("p t a b c -> p t (a b c)")
        else:
            qbd = consts.tile([P, NT, NPAIR, P], f16)
            qbd_z = qbd.rearrange("p t a b -> p t (a b)")
        # zero-fill the stationary slots up front on ACT via the uint32
        # bitcast (MEMSET runs at 1x on DVE; this is ~7x cheaper and keeps
        # DVE/GpSimd free for the diagonal writes and stream gates)
        for t in range(NT):
            nc.scalar.memzero(qbd_z[:, t, :])
        ident = consts.tile([P, P], f32)
        make_identity(nc, ident[:])
        ones_col = consts.tile([1, P], f16)
        nc.vector.memset(ones_col, 1.0)
        ident16 = consts.tile([P, P], f16)
        nc.vector.tensor_copy(ident16[:], ident[:])

        # ---- load small inputs (scalar ring, proj inputs first) ----
        x_sb = pro.tile([P, KT, N], f16)         # xT: partition = k % 128
        dma.dma_start(out=x_sb[:], in_=xT_ext.rearrange("(u p) n -> p u n", p=P))
        w_sb = pro.tile([P, KT, 3 * D], f16)
        dma2.dma_start(out=w_sb[:], in_=wqkv_ext.rearrange("(u p) n -> p u n", p=P))
        b_sb = pro.tile([1, 3 * D], f16)
        dma2.dma_start(out=b_sb[:], in_=bqkv_ext[:])
        rope_sb = pro.tile([P, NT, D], f32)
        dma2.dma_start(out=rope_sb[:], in_=rope_ext.rearrange("(t p) d -> p t d", p=P))
        wo_sb = consts.tile([D, DIM], f16)
        dma2.dma_start(out=wo_sb[:], in_=wo_ext[:])

        # q'^T / k'^T with an extra row 64: ones (q side) x additive mask row
        # (k side) so one k=65 matmul computes QK^T + mask bias.
        qpT16 = consts.tile([D + 1, N], f16)
        kpT16 = consts.tile([D + 1, N], f16)
        nc.vector.memset(qpT16[D:D + 1, :], 1.0)
        masku8 = pro.tile([D + 1, N], u8)
        dma2.dma_start(out=masku8[D:D + 1, :], in_=mask_ext[:])
        maskf = pro.tile([D + 1, N], f32)
        nc.vector.tensor_copy(maskf[D:D + 1, :], masku8[D:D + 1, :])
        nc.vector.tensor_scalar(kpT16[D:D + 1, :], maskf[D:D + 1, :],
                                MASK_BIG, -MASK_BIG, ALU.mult, ALU.add)

        # ---- cos/sin (ACT): cos(x) = sin(x + pi/2); fold QK scale into k's ----
        sin_sb = pro.tile([P, NT, D], f32)
        cos_sb = pro.tile([P, NT, D], f32)
        halfpi = consts.tile([P, 1], f32)
        nc.vector.memset(halfpi, float(np.pi / 2))
        nc.scalar.activation(sin_sb[:], rope_sb[:], AF.Sin)
        nc.scalar.activation(cos_sb[:], rope_sb[:], AF.Sin, bias=halfpi[:])
        kscale = SCALE * (AMP if FP8 else 1.0)
        sink_sb = pro.tile([P, NT, D], f32)
        cosk_sb = pro.tile([P, NT, D], f32)
        nc.vector.tensor_scalar_mul(sink_sb[:], sin_sb[:], kscale)
        nc.vector.tensor_scalar_mul(cosk_sb[:], cos_sb[:], kscale)

        # ---- per-row-tile prologue: q/k/v proj -> RoPE -> transposes ----
        q_sb = pro.tile([P, NT, D], f32)
        k_sb = pro.tile([P, NT, D], f32)
        v_sb = consts.tile([P, NT, D], f16)
        qp_sb = pro.tile([P, NT, D], f32)
        kp_sb = pro.tile([P, NT, D], f32)
        qbd_flat = qbd.rearrange("p t a b c -> p (t a b c)" if FP8 else
                                 "p t a b -> p (t a b)")
        for t in range(NT):
            ps = psum_p.tile([P, 3 * D], f32, tag="proj")
            for u in range(KT):
                nc.tensor.matmul(ps[:], x_sb[:, u, t * P:(t + 1) * P],
                                 w_sb[:, u, :], start=(u == 0), stop=False)
            nc.tensor.matmul(ps[:], ones_col[:, 0:P], b_sb[:],
                             start=False, stop=True)
            nc.scalar.copy(q_sb[:, t, :], ps[:, 0:D])
            nc.scalar.copy(k_sb[:, t, :], ps[:, D:2 * D])
            nc.scalar.copy(v_sb[:, t, :], ps[:, 2 * D:3 * D])
            for (src, dst, c, s) in ((q_sb, qp_sb, cos_sb, sin_sb),
                                     (k_sb, kp_sb, cosk_sb, sink_sb)):
                sr = src[:, t, :].rearrange("p (m two) -> p m two", two=2)
                rot = pro.tile([P, D // 2, 2], f32, tag="rot")
                nc.vector.tensor_scalar_mul(rot[:, :, 0], sr[:, :, 1], -1.0)
                nc.vector.tensor_copy(rot[:, :, 1], sr[:, :, 0])
                tmp = pro.tile([P, D], f32, tag="ropetmp")
                nc.vector.tensor_mul(tmp[:], rot.rearrange("p m two -> p (m two)"),
                                     s[:, t, :])
                nc.vector.tensor_mul(dst[:, t, :], src[:, t, :], c[:, t, :])
                nc.vector.tensor_add(dst[:, t, :], dst[:, t, :], tmp[:])
            # full q' transpose (64, 128) at psum base 0
            ps1 = psum_t.tile([P, P], f32, tag="tp")
            nc.tensor.transpose(ps1[0:D, :], qp_sb[:, t, :], ident[:])
            nc.scalar.copy(qpT16[0:D, t * P:(t + 1) * P], ps1[0:D, :])
            # half transpose of q' rows 0:64 (even group) at psum base 64.
            # Transpose-mode matmuls must write psum partition 0, so use a
            # regular matmul (qp16 as stationary x identity) col-tiled to 64.
            qp16 = pro.tile([D, D], f16, tag="qp16")
            nc.vector.tensor_copy(qp16[:], qp_sb[0:D, t, :])
            ps2 = psum_t.tile([P, P], f32, tag="tp")
            nc.tensor.matmul(ps2[D:P, 0:D], qp16[:], ident16[0:D, 0:D],
                             start=True, stop=True)
            if FP8:
                # quad-stationary diagonals (x Q_AMP, f32->fp8 strided):
                # quad qd covers i_locals {qd+64, qd, qd+96, qd+32} at
                # (ks0 k-lo, ks0 k-hi, ks1 k-lo, ks1 k-hi), cols = i_local
                base = t * NQ * 2 * P
                st = 2 * P + 1
                top = base + NQ * 2 * P
                for off, dst_lo, s0, s1 in ((D, True, D, D + NQ),
                                            (0, False, 0, NQ),
                                            (3 * D + NQ, True, 3 * NQ, P),
                                            (2 * D + NQ, False, NQ, 2 * NQ)):
                    srcp = ps1 if dst_lo else ps2
                    dst = (qbd_flat[0:D, base + off:top:st] if dst_lo else
                           qbd_flat[D:P, base + off:top:st])
                    nc.vector.tensor_scalar_mul(
                        dst, (srcp[0:D, s0:s1] if dst_lo else
                              srcp[D:P, s0:s1]), Q_AMP)
            else:
                # pair-stationary diagonals (f32->f16 strided):
                # col p+64 (k 0:64) <- q'[base+64+p]; col p (k 64:128) <- q'[base+p]
                base = t * NPAIR * P
                nc.vector.tensor_copy(
                    qbd_flat[0:D, base + D:base + NPAIR * P:P + 1],
                    ps1[0:D, D:P])
                nc.vector.tensor_copy(
                    qbd_flat[D:P, base:base + NPAIR * P:P + 1],
                    ps2[D:P, 0:D])
            # k' transpose
            ps3 = psum_t.tile([P, P], f32, tag="tp")
            nc.tensor.transpose(ps3[0:D, :], kp_sb[:, t, :], ident[:])
            nc.scalar.copy(kpT16[0:D, t * P:(t + 1) * P], ps3[0:D, :])

        # ---- main loop over row tiles ----
        def softmax_av_out(it, dots_ps):
            unscale = 1.0 / AMP if FP8 else 1.0
            # no row-max pass: |dots| <= ~10 so exp() is f32-safe, and masked
            # lanes sit at <= -100 where exp underflows to 0 as required
            w_sm = sm.tile([P, N], f32, tag="w_sm")
            rowsum = sm.tile([P, 1], f32, tag="rowsum")
            nc.scalar.activation(w_sm[:], dots_ps[:], AF.Exp, scale=unscale,
                                 accum_out=rowsum[:])
            rcp = sm.tile([P, 1], f32, tag="rcp")
            nc.vector.reciprocal(rcp[:], rowsum[:])
            wT16 = outp.tile([P, NT, P], f16, tag="wT16")
            for jt in range(NT):
                wT_ps = psum_t.tile([P, P], f32, tag="tp")
                nc.tensor.transpose(wT_ps[:], w_sm[:, jt * P:(jt + 1) * P],
                                    ident[:])
                nc.scalar.copy(wT16[:, jt, :], wT_ps[:])
            attn_ps = psum_av.tile([D, P], f32, tag="attn")
            for jt in range(NT):
                nc.tensor.matmul(attn_ps[:], v_sb[:, jt, :], wT16[:, jt, :],
                                 start=(jt == 0), stop=(jt == NT - 1))
            attn16 = outp.tile([D, P], f16, tag="attn16")
            nc.scalar.copy(attn16[:], attn_ps[:])
            out_ps = psum_o.tile([P, DIM], f32, tag="out_ps")
            nc.tensor.matmul(out_ps[:], attn16[:], wo_sb[:], start=True, stop=True)
            # softmax normalization folded in here: rows scale by 1/rowsum
            o_sb = outp.tile([P, DIM], f16, tag="o_sb")
            nc.scalar.mul(o_sb[:], out_ps[:], rcp[:])
            dma2.dma_start(out=out_ext[it * P:(it + 1) * P, :], in_=o_sb[:])

        # ring A (sync) carries x then even rel groups; ring B (scalar) the
        # other inputs then odd rel groups. All 8 group DMAs are issued UP
        # FRONT with one SBUF buffer each: ring B's dma_starts otherwise sit
        # behind the ACT engine's softmax copies and straggle. Gate every
        # group on its ring's last input so no rel DMA overtakes them.
        gate_a = x_sb[0:1, 0, 0:1]
        gate_b = masku8[D:D + 1, 0:1]
        NGROUP = NGQ if FP8 else NG
        rl_tiles = []
        for gidx in range(NT * NGROUP):
            if FP8:
                rl = relp.tile([P, GQ, 2, N], f8)
            else:
                rl = relp.tile([P, G, N], f16)
            nc.gpsimd.tensor_copy(
                rl.rearrange("p a b -> p (a b)" if not FP8 else
                             "p a b c -> p (a b c)")[0:1, 0:1],
                gate_a if gidx % 2 == 0 else gate_b)
            ring = dma if gidx % 2 == 0 else dma2
            ring.dma_start(out=rl[:], in_=rel_ext[gidx // NGROUP, gidx % NGROUP])
            rl_tiles.append(rl)
        for it in range(NT):
            dots_ps = psum_mm.tile([P, N], f32, tag="mm")
            # rel matmuls first (start=True on the first clears the bank),
            # QK^T + mask joins last so it never gates the stream.
            for gr in range(NGROUP):
                rl = rl_tiles[it * NGROUP + gr]
                if FP8:
                    for g8 in range(GQ):
                        qd = gr * GQ + g8
                        nc.tensor.matmul(dots_ps[:], qbd[:, it, qd, :, :],
                                         rl[:, g8, :, :], start=(qd == 0),
                                         stop=False,
                                         perf_mode=mybir.MatmulPerfMode.DoubleRow)
                else:
                    for g8 in range(G):
                        pr = gr * G + g8
                        nc.tensor.matmul(dots_ps[:], qbd[:, it, pr, :],
                                         rl[:, g8, :], start=(pr == 0),
                                         stop=False)
            nc.tensor.matmul(dots_ps[:], qpT16[:, it * P:(it + 1) * P], kpT16[:],
                             start=False, stop=True)
            softmax_av_out(it, dots_ps)

    legalize_multi_waits(nc)
    return nc


_NC_CACHE = None
TRACE = False        # set by test harness to capture an NTFF profile
LAST_RESULT = None   # BassKernelResults of the most recent kernel() call


def _get_nc():
    global _NC_CACHE
    if _NC_CACHE is None:
        _NC_CACHE = build_nc()
    return _NC_CACHE


def _repack_rel(rel_h):
    """(N, N, D) f32 -> (NT, NG, 2*D, G, N) fp16 pair layout: each (it, gr)
    DMA group is one fully contiguous DRAM block, partition k major inside;
    k rows 0:64 hold d of i1=base+64+p (odd group), 64:128 of i0=base+p."""
    r = rel_h.transpose(0, 2, 1)                       # (i, d, j)
    rh = r.reshape(NT, 2, NPAIR, D, N)                 # (it, g, p, d, j)
    a = rh[:, ::-1].transpose(0, 1, 3, 2, 4).reshape(NT, P, NPAIR, N)
    a = a.reshape(NT, P, NG, G, N).transpose(0, 2, 1, 3, 4)
    return np.ascontiguousarray(a, dtype=np.float16)


def _repack_rel_fp8(rel_h):
    """(N, N, D) f32 -> (NT, NGQ, 2*D, GQ, 2, N) e4m3 quad layout (x REL_AMP):
    quad q covers i_locals {q+64, q, q+96, q+32} on (ks, k-half) slots
    (0,lo), (0,hi), (1,lo), (1,hi); each (it, gr) block is contiguous."""
    r = rel_h.transpose(0, 2, 1)                       # (i, d, j)
    b5 = r.reshape(NT, 4, NQ, D, N)                    # blk = i_local // 32
    b6 = b5[:, [2, 0, 3, 1]]                           # s = 2*ks + half
    a = b6.reshape(NT, 2, 2, NQ, D, N).transpose(0, 2, 4, 3, 1, 5)
    a = a.reshape(NT, P, NQ, 2, N)
    a = a.reshape(NT, P, NGQ, GQ, 2, N).transpose(0, 2, 1, 3, 4, 5)
    import ml_dtypes
    return np.ascontiguousarray((a * REL_AMP).astype(ml_dtypes.float8_e4m3))


def kernel(**inputs):
    x = np.asarray(inputs["x"], dtype=np.float32)
    mask = np.asarray(inputs["mask"])
    rope = np.asarray(inputs["rope"], dtype=np.float32)
    rel_pos = np.asarray(inputs["rel_pos"], dtype=np.float32)
    Wq = np.asarray(inputs["Wq"], dtype=np.float32)
    bq = np.asarray(inputs["bq"], dtype=np.float32)
    Wk = np.asarray(inputs["Wk"], dtype=np.float32)
    bk = np.asarray(inputs["bk"], dtype=np.float32)
    Wv = np.asarray(inputs["Wv"], dtype=np.float32)
    bv = np.asarray(inputs["bv"], dtype=np.float32)
    Wo = np.asarray(inputs["Wo"], dtype=np.float32)
    bo = np.asarray(inputs["bo"], dtype=np.float32)

    nc = _get_nc()

    xT = np.ascontiguousarray(x.reshape(N, DIM).T).astype(np.float16)
    mask_u8 = np.ascontiguousarray(mask.reshape(1, N).astype(np.uint8, copy=False))
    rope2 = np.ascontiguousarray(rope)

    in_maps = []
    for h in range(N_CORES):
        sl = slice(h * D, (h + 1) * D)
        wqkv = np.concatenate([Wq[:, sl], Wk[:, sl], Wv[:, sl]],
                              axis=1).astype(np.float16)
        bqkv = np.concatenate([bq[sl], bk[sl], bv[sl]])[None, :].astype(np.float16)
        in_maps.append({
            "xT": xT,
            "wqkv": np.ascontiguousarray(wqkv),
            "bqkv": np.ascontiguousarray(bqkv),
            "wo": np.ascontiguousarray(Wo[sl, :]).astype(np.float16),
            "rope": rope2,
            "mask": mask_u8,
            "rel": (_repack_rel_fp8 if FP8 else _repack_rel)(rel_pos[0, h]),
        })

    from concourse.bass_utils import run_bass_kernel_spmd
    res = run_bass_kernel_spmd(nc, in_maps, list(range(N_CORES)), trace=TRACE)
    globals()["LAST_RESULT"] = res
    out = np.zeros((N, DIM), dtype=np.float32)
    for h in range(N_CORES):
        out += res.results[h]["out"].astype(np.float32)
    out += bo[None, :]
    return out.reshape(B, N, DIM)


# revision 37
# speedup vs baseline: 1.2077x; 1.1082x over previous
"""Bass/Trainium2 kernel for nn_Attention_66297115181568 (sparse_attention).

Strategy: head-parallel across 8 NeuronCores. Core h computes head h
end-to-end; the host sums the 8 partial (512, 512) outputs (the
tensor-parallel all-reduce done at unshard time) and adds bo.

The dominant cost is streaming rel_pos (67MB/core as f32). Key ideas:
  1. rel_pos is scaled by 64, cast to fp8 e4m3, and repacked ON THE HOST
     into a quad layout: DMA bytes drop 4x to 16.8MB/core, streamed as
     2MB fully-contiguous DRAM blocks alternating across both HWDGE
     rings (~410GB/s combined).
  2. The rel contraction relterm[i,j] = sum_d q'[i,d]*rel[i,j,d] runs
     on the tensor engine as DoubleRow fp8 matmuls: for each quad of
     rows {q+64, q, q+96, q+32} a (128, 2, 128) stationary holds the
     four q' vectors (x8) on its block-diagonal, zeros elsewhere; the
     moving operand stacks the four rows' rel d-vectors on (partition,
     k-subtile). 32 such matmuls accumulate 512*relterm directly into
     the QK^T PSUM tile (zero columns add nothing) -- no separate
     relterm buffer, elementwise products, or reductions exist.
  3. QK^T (+ additive mask via an extra k=65 ones/mask row) is scaled
     by 512 to match, and one exp(x/512) on ACT undoes it; softmax
     skips the row-max pass (logits are bounded, masked lanes sit at
     <= -100 and underflow to exactly 0) and the 1/rowsum normalization
     is folded into the output copy.
  4. The block-diagonal stationary zeros are written via ACT uint32
     bitcast memzero (MEMSET runs at 1x on DVE and would gate the
     pipeline); only the 512 diagonal values are rewritten per tile by
     strided DVE casts.

Per-core engine plan:
  PE    : q/k/v projections (fp16), q'/k' transposes, QK^T+mask, 32
          DoubleRow rel matmuls per row tile, softmax-weight
          transposes, AV, Wo partial.
  ACT   : stationary memzero, PSUM->SBUF copies (f32->fp16 casts), exp
          with fused row-sum, normalized output copy.
  DVE   : RoPE, block-diagonal strided diagonal writes, reciprocal.
  GpSimd: identity build + stream-gate touches.
  DMA   : fp8 rel stream on both rings behind the small inputs, fully
          overlapped with compute.
"""

import sys

sys.path.insert(0, "/opt/trn_rl_repo")

from contextlib import ExitStack

import numpy as np

import concourse.bass as bass
import concourse.tile as tile
from concourse import mybir
from concourse.masks import make_identity

# problem dims (hardcoded per spec)
B, N, DIM, H, D = 1, 512, 512, 8, 64
INNER = H * D
N_CORES = 8
P = 128                 # SBUF partitions
NT = N // P             # 4 row tiles
KT = DIM // P           # 4 contraction tiles
NPAIR = P // 2          # 64 row pairs per row tile
G = 16                  # pairs per rel DMA (128p x 16KB = 2MB)
NG = NPAIR // G         # 4 DMA groups per row tile
FP8 = True              # stream rel as fp8 e4m3 + DoubleRow quad matmuls
NQ = P // 4             # 32 row quads per row tile (fp8 path)
GQ = 16                 # quads per rel DMA (128p x 16KB = 2MB)
NGQ = NQ // GQ          # 2 DMA groups per row tile (fp8 path)
REL_AMP = 64.0          # host premultiplier on rel (fp8 range use)
Q_AMP = 8.0             # on-device premultiplier on q' diagonals (fp8)
AMP = REL_AMP * Q_AMP   # net scale on dots, undone in the exp
SCALE = D ** -0.5
MASK_BIG = 60000.0      # fp16-safe additive mask magnitude

f32 = mybir.dt.float32
f16 = mybir.dt.float16
f8 = mybir.dt.float8e4
u8 = mybir.dt.uint8
AX = mybir.AxisListType
ALU = mybir.AluOpType
AF = mybir.ActivationFunctionType


def legalize_multi_waits(nc):
    """This walrus build supports only one sync-wait per instruction; hoist
    extra waits onto same-engine NoOps placed immediately before."""
    nid = 0
    for fn in nc.m.functions:
        for bb in fn.blocks:
            new = []
            changed = False
            for inst in bb.instructions:
                si = inst.sync_info
                waits = si.on_wait if si is not None else []
                if len(waits) > 1:
                    for w in waits[:-1]:
                        nop = mybir.InstNoOp(name=f"I-waitfix-{nid}")
                        nid += 1
                        nop.engine = inst.engine
                        nop.sync_info = mybir.SyncInfo(on_wait=[w], on_update=[])
                        new.append(nop)
                    si.on_wait = [waits[-1]]
                    inst.sync_info = si
                    changed = True
                new.append(inst)
            if changed:
                bb.instructions = new


def build_nc():
    nc = bass.Bass()

    xT_ext = nc.declare_dram_parameter("xT", [DIM, N], f16, isOutput=False)
    wqkv_ext = nc.declare_dram_parameter("wqkv", [DIM, 3 * D], f16, isOutput=False)
    bqkv_ext = nc.declare_dram_parameter("bqkv", [1, 3 * D], f16, isOutput=False)
    wo_ext = nc.declare_dram_parameter("wo", [D, DIM], f16, isOutput=False)
    rope_ext = nc.declare_dram_parameter("rope", [N, D], f32, isOutput=False)
    mask_ext = nc.declare_dram_parameter("mask", [1, N], u8, isOutput=False)
    if FP8:
        rel_ext = nc.declare_dram_parameter("rel", [NT, NGQ, P, GQ, 2, N], f8,
                                            isOutput=False)
    else:
        rel_ext = nc.declare_dram_parameter("rel", [NT, NG, P, G, N], f16,
                                            isOutput=False)
    out_ext = nc.declare_dram_parameter("out", [N, DIM], f16, isOutput=True)

    with tile.TileContext(nc) as tc, ExitStack() as ctx:
        dma = nc.sync      # HWDGE; inputs first, then the rel_pos stream
        dma2 = nc.scalar   # HWDGE; outputs (kept off the rel stream ring)
        consts = ctx.enter_context(tc.tile_pool(name="consts", bufs=1))
        # PSUM: 8 banks -- 2 transpose + 2 dots + 1 av + 2 out/proj
        psum_t = ctx.enter_context(
            tc.tile_pool(name="psum_t", bufs=2, space=bass.MemorySpace.PSUM))
        psum_mm = ctx.enter_context(
            tc.tile_pool(name="psum_mm", bufs=2, space=bass.MemorySpace.PSUM))
        psum_av = ctx.enter_context(
            tc.tile_pool(name="psum_av", bufs=1, space=bass.MemorySpace.PSUM))
        psum_o = ctx.enter_context(
            tc.tile_pool(name="psum_o", bufs=1, space=bass.MemorySpace.PSUM))
        psum_p = ctx.enter_context(
            tc.tile_pool(name="psum_p", bufs=2, space=bass.MemorySpace.PSUM))
        pro = ctx.enter_context(tc.tile_pool(name="pro", bufs=1))
        relp = ctx.enter_context(tc.tile_pool(name="relp", bufs=8))
        sm = ctx.enter_context(tc.tile_pool(name="sm", bufs=2))
        outp = ctx.enter_context(tc.tile_pool(name="outp", bufs=2))

        # ---- constants ----
        # block-diagonal stationaries: qbd[k, it, pair, col]; zeros persist,
        # only the two diagonals are rewritten per row tile. Tile 0's zeros
        # are the first DVE op (critical path); tiles 1-3 are zeroed inside
        # the prologue loop so they overlap the rel stream.
        if FP8:
            qbd = consts.tile([P, NT, NQ, 2, P], f8)
            qbd_z = qbd.rearrange("p t a b c -> p t (a b c)")
        else:
            qbd = consts.tile([P, NT, NPAIR, P], f16)
            qbd_z = qbd.rearrange("p t a b -> p t (a b)")
        # zero-fill the stationary slots up front on ACT via the uint32
        # bitcast (MEMSET runs at 1x on DVE; this is ~7x cheaper and keeps
        # DVE/GpSimd free for the diagonal writes and stream gates)
        for t in range(NT):
            nc.scalar.memzero(qbd_z[:, t, :])
        ident = consts.tile([P, P], f32)
        make_identity(nc, ident[:])
        ones_col = consts.tile([1, P], f16)
        nc.vector.memset(ones_col, 1.0)
        ident16 = consts.tile([P, P], f16)
        nc.vector.tensor_copy(ident16[:], ident[:])

        # ---- load small inputs (scalar ring, proj inputs first) ----
        x_sb = pro.tile([P, KT, N], f16)         # xT: partition = k % 128
        dma.dma_start(out=x_sb[:], in_=xT_ext.rearrange("(u p) n -> p u n", p=P))
        w_sb = pro.tile([P, KT, 3 * D], f16)
        dma2.dma_start(out=w_sb[:], in_=wqkv_ext.rearrange("(u p) n -> p u n", p=P))
        b_sb = pro.tile([1, 3 * D], f16)
        dma2.dma_start(out=b_sb[:], in_=bqkv_ext[:])
        rope_sb = pro.tile([P, NT, D], f32)
        dma2.dma_start(out=rope_sb[:], in_=rope_ext.rearrange("(t p) d -> p t d", p=P))
        wo_sb = consts.tile([D, DIM], f16)
        dma2.dma_start(out=wo_sb[:], in_=wo_ext[:])

        # q'^T / k'^T with an extra row 64: ones (q side) x additive mask row
        # (k side) so one k=65 matmul computes QK^T + mask bias.
        qpT16 = consts.tile([D + 1, N], f16)
        kpT16 = consts.tile([D + 1, N], f16)
        nc.vector.memset(qpT16[D:D + 1, :], 1.0)
        masku8 = pro.tile([D + 1, N], u8)
        dma2.dma_start(out=masku8[D:D + 1, :], in_=mask_ext[:])
        maskf = pro.tile([D + 1, N], f32)
        nc.vector.tensor_copy(maskf[D:D + 1, :], masku8[D:D + 1, :])
        nc.vector.tensor_scalar(kpT16[D:D + 1, :], maskf[D:D + 1, :],
                                MASK_BIG, -MASK_BIG, ALU.mult, ALU.add)

        # ring A (sync) carries x then even rel groups; ring B (scalar) the
        # other inputs then odd rel groups. All 8 group DMAs are issued UP
        # FRONT with one SBUF buffer each: ring B's dma_starts otherwise sit
        # behind the ACT engine's softmax copies and straggle. Gate every
        # group on its ring's last input so no rel DMA overtakes them.
        gate_a = x_sb[0:1, 0, 0:1]
        gate_b = masku8[D:D + 1, 0:1]
        NGROUP = NGQ if FP8 else NG
        rl_tiles = []
        for gidx in range(NT * NGROUP):
            if FP8:
                rl = relp.tile([P, GQ, 2, N], f8)
            else:
                rl = relp.tile([P, G, N], f16)
            nc.gpsimd.tensor_copy(
                rl.rearrange("p a b -> p (a b)" if not FP8 else
                             "p a b c -> p (a b c)")[0:1, 0:1],
                gate_a if gidx % 2 == 0 else gate_b)
            ring = dma if gidx % 2 == 0 else dma2
            ring.dma_start(out=rl[:], in_=rel_ext[gidx // NGROUP, gidx % NGROUP])
            rl_tiles.append(rl)

        # ---- cos/sin (ACT): cos(x) = sin(x + pi/2); fold QK scale into k's ----
        sin_sb = pro.tile([P, NT, D], f32)
        cos_sb = pro.tile([P, NT, D], f32)
        halfpi = consts.tile([P, 1], f32)
        nc.vector.memset(halfpi, float(np.pi / 2))
        nc.scalar.activation(sin_sb[:], rope_sb[:], AF.Sin)
        nc.scalar.activation(cos_sb[:], rope_sb[:], AF.Sin, bias=halfpi[:])
        kscale = SCALE * (AMP if FP8 else 1.0)
        sink_sb = pro.tile([P, NT, D], f32)
        cosk_sb = pro.tile([P, NT, D], f32)
        nc.vector.tensor_scalar_mul(sink_sb[:], sin_sb[:], kscale)
        nc.vector.tensor_scalar_mul(cosk_sb[:], cos_sb[:], kscale)

        # ---- per-row-tile prologue: q/k/v proj -> RoPE -> transposes ----
        q_sb = pro.tile([P, NT, D], f32)
        k_sb = pro.tile([P, NT, D], f32)
        v_sb = consts.tile([P, NT, D], f16)
        qp_sb = pro.tile([P, NT, D], f32)
        kp_sb = pro.tile([P, NT, D], f32)
        qbd_flat = qbd.rearrange("p t a b c -> p (t a b c)" if FP8 else
                                 "p t a b -> p (t a b)")
        for t in range(NT):
            ps = psum_p.tile([P, 3 * D], f32, tag="proj")
            for u in range(KT):
                nc.tensor.matmul(ps[:], x_sb[:, u, t * P:(t + 1) * P],
                                 w_sb[:, u, :], start=(u == 0), stop=False)
            nc.tensor.matmul(ps[:], ones_col[:, 0:P], b_sb[:],
                             start=False, stop=True)
            nc.scalar.copy(q_sb[:, t, :], ps[:, 0:D])
            nc.scalar.copy(k_sb[:, t, :], ps[:, D:2 * D])
            nc.scalar.copy(v_sb[:, t, :], ps[:, 2 * D:3 * D])
            for (src, dst, c, s) in ((q_sb, qp_sb, cos_sb, sin_sb),
                                     (k_sb, kp_sb, cosk_sb, sink_sb)):
                sr = src[:, t, :].rearrange("p (m two) -> p m two", two=2)
                rot = pro.tile([P, D // 2, 2], f32, tag="rot")
                nc.vector.tensor_scalar_mul(rot[:, :, 0], sr[:, :, 1], -1.0)
                nc.vector.tensor_copy(rot[:, :, 1], sr[:, :, 0])
                tmp = pro.tile([P, D], f32, tag="ropetmp")
                nc.vector.tensor_mul(tmp[:], rot.rearrange("p m two -> p (m two)"),
                                     s[:, t, :])
                nc.vector.tensor_mul(dst[:, t, :], src[:, t, :], c[:, t, :])
                nc.vector.tensor_add(dst[:, t, :], dst[:, t, :], tmp[:])
            # full q' transpose (64, 128) at psum base 0
            ps1 = psum_t.tile([P, P], f32, tag="tp")
            nc.tensor.transpose(ps1[0:D, :], qp_sb[:, t, :], ident[:])
            nc.scalar.copy(qpT16[0:D, t * P:(t + 1) * P], ps1[0:D, :])
            # half transpose of q' rows 0:64 (even group) at psum base 64.
            # Transpose-mode matmuls must write psum partition 0, so use a
            # regular matmul (qp16 as stationary x identity) col-tiled to 64.
            qp16 = pro.tile([D, D], f16, tag="qp16")
            nc.vector.tensor_copy(qp16[:], qp_sb[0:D, t, :])
            ps2 = psum_t.tile([P, P], f32, tag="tp")
            nc.tensor.matmul(ps2[D:P, 0:D], qp16[:], ident16[0:D, 0:D],
                             start=True, stop=True)
            if FP8:
                # quad-stationary diagonals (x Q_AMP, f32->fp8 strided):
                # quad qd covers i_locals {qd+64, qd, qd+96, qd+32} at
                # (ks0 k-lo, ks0 k-hi, ks1 k-lo, ks1 k-hi), cols = i_local
                base = t * NQ * 2 * P
                st = 2 * P + 1
                top = base + NQ * 2 * P
                for off, dst_lo, s0, s1 in ((D, True, D, D + NQ),
                                            (0, False, 0, NQ),
                                            (3 * D + NQ, True, 3 * NQ, P),
                                            (2 * D + NQ, False, NQ, 2 * NQ)):
                    srcp = ps1 if dst_lo else ps2
                    dst = (qbd_flat[0:D, base + off:top:st] if dst_lo else
                           qbd_flat[D:P, base + off:top:st])
                    nc.vector.tensor_scalar_mul(
                        dst, (srcp[0:D, s0:s1] if dst_lo else
                              srcp[D:P, s0:s1]), Q_AMP)
            else:
                # pair-stationary diagonals (f32->f16 strided):
                # col p+64 (k 0:64) <- q'[base+64+p]; col p (k 64:128) <- q'[base+p]
                base = t * NPAIR * P
                nc.vector.tensor_copy(
                    qbd_flat[0:D, base + D:base + NPAIR * P:P + 1],
                    ps1[0:D, D:P])
                nc.vector.tensor_copy(
                    qbd_flat[D:P, base:base + NPAIR * P:P + 1],
                    ps2[D:P, 0:D])
            # k' transpose
            ps3 = psum_t.tile([P, P], f32, tag="tp")
            nc.tensor.transpose(ps3[0:D, :], kp_sb[:, t, :], ident[:])
            nc.scalar.copy(kpT16[0:D, t * P:(t + 1) * P], ps3[0:D, :])

        # ---- main loop over row tiles ----
        def softmax_av_out(it, dots_ps):
            unscale = 1.0 / AMP if FP8 else 1.0
            # no row-max pass: |dots| <= ~10 so exp() is f32-safe, and masked
            # lanes sit at <= -100 where exp underflows to 0 as required
            w_sm = sm.tile([P, N], f32, tag="w_sm")
            rowsum = sm.tile([P, 1], f32, tag="rowsum")
            nc.scalar.activation(w_sm[:], dots_ps[:], AF.Exp, scale=unscale,
                                 accum_out=rowsum[:])
            rcp = sm.tile([P, 1], f32, tag="rcp")
            nc.vector.reciprocal(rcp[:], rowsum[:])
            wT16 = outp.tile([P, NT, P], f16, tag="wT16")
            for jt in range(NT):
                wT_ps = psum_t.tile([P, P], f32, tag="tp")
                nc.tensor.transpose(wT_ps[:], w_sm[:, jt * P:(jt + 1) * P],
                                    ident[:])
                nc.scalar.copy(wT16[:, jt, :], wT_ps[:])
            attn_ps = psum_av.tile([D, P], f32, tag="attn")
            for jt in range(NT):
                nc.tensor.matmul(attn_ps[:], v_sb[:, jt, :], wT16[:, jt, :],
                                 start=(jt == 0), stop=(jt == NT - 1))
            attn16 = outp.tile([D, P], f16, tag="attn16")
            nc.scalar.copy(attn16[:], attn_ps[:])
            out_ps = psum_o.tile([P, DIM], f32, tag="out_ps")
            nc.tensor.matmul(out_ps[:], attn16[:], wo_sb[:], start=True, stop=True)
            # softmax normalization folded in here: rows scale by 1/rowsum
            o_sb = outp.tile([P, DIM], f16, tag="o_sb")
            nc.scalar.mul(o_sb[:], out_ps[:], rcp[:])
            dma2.dma_start(out=out_ext[it * P:(it + 1) * P, :], in_=o_sb[:])

        for it in range(NT):
            dots_ps = psum_mm.tile([P, N], f32, tag="mm")
            # rel matmuls first (start=True on the first clears the bank),
            # QK^T + mask joins last so it never gates the stream.
            for gr in range(NGROUP):
                rl = rl_tiles[it * NGROUP + gr]
                if FP8:
                    for g8 in range(GQ):
                        qd = gr * GQ + g8
                        nc.tensor.matmul(dots_ps[:], qbd[:, it, qd, :, :],
                                         rl[:, g8, :, :], start=(qd == 0),
                                         stop=False,
                                         perf_mode=mybir.MatmulPerfMode.DoubleRow)
                else:
                    for g8 in range(G):
                        pr = gr * G + g8
                        nc.tensor.matmul(dots_ps[:], qbd[:, it, pr, :],
                                         rl[:, g8, :], start=(pr == 0),
                                         stop=False)
            nc.tensor.matmul(dots_ps[:], qpT16[:, it * P:(it + 1) * P], kpT16[:],
                             start=False, stop=True)
            softmax_av_out(it, dots_ps)

    legalize_multi_waits(nc)
    return nc


_NC_CACHE = None
TRACE = False        # set by test harness to capture an NTFF profile
LAST_RESULT = None   # BassKernelResults of the most recent kernel() call


def _get_nc():
    global _NC_CACHE
    if _NC_CACHE is None:
        _NC_CACHE = build_nc()
    return _NC_CACHE


def _repack_rel(rel_h):
    """(N, N, D) f32 -> (NT, NG, 2*D, G, N) fp16 pair layout: each (it, gr)
    DMA group is one fully contiguous DRAM block, partition k major inside;
    k rows 0:64 hold d of i1=base+64+p (odd group), 64:128 of i0=base+p."""
    r = rel_h.transpose(0, 2, 1)                       # (i, d, j)
    rh = r.reshape(NT, 2, NPAIR, D, N)                 # (it, g, p, d, j)
    a = rh[:, ::-1].transpose(0, 1, 3, 2, 4).reshape(NT, P, NPAIR, N)
    a = a.reshape(NT, P, NG, G, N).transpose(0, 2, 1, 3, 4)
    return np.ascontiguousarray(a, dtype=np.float16)


def _repack_rel_fp8(rel_h):
    """(N, N, D) f32 -> (NT, NGQ, 2*D, GQ, 2, N) e4m3 quad layout (x REL_AMP):
    quad q covers i_locals {q+64, q, q+96, q+32} on (ks, k-half) slots
    (0,lo), (0,hi), (1,lo), (1,hi); each (it, gr) block is contiguous."""
    r = rel_h.transpose(0, 2, 1)                       # (i, d, j)
    b5 = r.reshape(NT, 4, NQ, D, N)                    # blk = i_local // 32
    b6 = b5[:, [2, 0, 3, 1]]                           # s = 2*ks + half
    a = b6.reshape(NT, 2, 2, NQ, D, N).transpose(0, 2, 4, 3, 1, 5)
    a = a.reshape(NT, P, NQ, 2, N)
    a = a.reshape(NT, P, NGQ, GQ, 2, N).transpose(0, 2, 1, 3, 4, 5)
    import ml_dtypes
    return np.ascontiguousarray((a * REL_AMP).astype(ml_dtypes.float8_e4m3))


def kernel(**inputs):
    x = np.asarray(inputs["x"], dtype=np.float32)
    mask = np.asarray(inputs["mask"])
    rope = np.asarray(inputs["rope"], dtype=np.float32)
    rel_pos = np.asarray(inputs["rel_pos"], dtype=np.float32)
    Wq = np.asarray(inputs["Wq"], dtype=np.float32)
    bq = np.asarray(inputs["bq"], dtype=np.float32)
    Wk = np.asarray(inputs["Wk"], dtype=np.float32)
    bk = np.asarray(inputs["bk"], dtype=np.float32)
    Wv = np.asarray(inputs["Wv"], dtype=np.float32)
    bv = np.asarray(inputs["bv"], dtype=np.float32)
    Wo = np.asarray(inputs["Wo"], dtype=np.float32)
    bo = np.asarray(inputs["bo"], dtype=np.float32)

    nc = _get_nc()

    xT = np.ascontiguousarray(x.reshape(N, DIM).T).astype(np.float16)
    mask_u8 = np.ascontiguousarray(mask.reshape(1, N).astype(np.uint8, copy=False))
    rope2 = np.ascontiguousarray(rope)

    in_maps = []
    for h in range(N_CORES):
        sl = slice(h * D, (h + 1) * D)
        wqkv = np.concatenate([Wq[:, sl], Wk[:, sl], Wv[:, sl]],
                              axis=1).astype(np.float16)
        bqkv = np.concatenate([bq[sl], bk[sl], bv[sl]])[None, :].astype(np.float16)
        in_maps.append({
            "xT": xT,
            "wqkv": np.ascontiguousarray(wqkv),
            "bqkv": np.ascontiguousarray(bqkv),
            "wo": np.ascontiguousarray(Wo[sl, :]).astype(np.float16),
            "rope": rope2,
            "mask": mask_u8,
            "rel": (_repack_rel_fp8 if FP8 else _repack_rel)(rel_pos[0, h]),
        })

    from concourse.bass_utils import run_bass_kernel_spmd
    res = run_bass_kernel_spmd(nc, in_maps, list(range(N_CORES)), trace=TRACE)
    globals()["LAST_RESULT"] = res
    out = np.zeros((N, DIM), dtype=np.float32)
    for h in range(N_CORES):
        out += res.results[h]["out"].astype(np.float32)
    out += bo[None, :]
    return out.reshape(B, N, DIM)
